# revision 3
# baseline (speedup 1.0000x reference)
"""Fully-fused fp16 MoE expert FFN (E=8, C=2048, D=1024, F=4096), 8 TRN2 cores.

One expert per core; w1 AND w2 SBUF-resident in fp16. v3 schedule:
- All DMA on the two HWDGE rings (sync + scalar); they drain concurrently
  at packet granularity so the pair saturates HBM. No SWDGE: its 16KB
  packets monopolize the shared SDMA engines and starve 1KB-packet loads.
- Critical path to the first matmul is only xt chunk-0 first half + w1
  cols 0-127 (~1MB): chunk-0 mm1 runs as two N=256 half-passes.
- w2 (8MB) streams on the scalar ring in 0.5MB pieces interleaved between
  chunk-0 pass-B gelus, landing just before chunk-0 mm2 consumes it.
- N=128 warm-up matmuls on a zeroed tile during the DMA head keep the HAM
  clock gate busy so the real stream starts at 2.4GHz.
- Final output group evicts in 4x128-col strips to shorten the tail.
"""

import numpy as np

import concourse.bass as bass
import concourse.mybir as mybir
import concourse.tile as tile
from concourse import bacc
from concourse.bass_utils import run_bass_kernel_spmd

E, C, D, F = 8, 2048, 1024, 4096
P = 128
KD = D // P  # 8
MF = F // P  # 32
CN = C // 512  # 4 chunks of 512 tokens
CJ = 4  # 128-token subblocks per chunk
DN = D // 512  # 2
NWARM = 30

F32 = mybir.dt.float32
F16 = mybir.dt.float16
GELU = mybir.ActivationFunctionType.Gelu_apprx_tanh
ds = bass.ds

_CACHE = {}


def _build():
    nc = bacc.Bacc("TRN2", target_bir_lowering=False, debug=False, num_devices=E)

    xT_d = nc.dram_tensor("xT", [P, KD, C], F16, kind="ExternalInput").ap()
    w1_d = nc.dram_tensor("w1r", [P, KD, F], F16, kind="ExternalInput").ap()
    b1_d = nc.dram_tensor("b1t", [P, MF], F32, kind="ExternalInput").ap()
    w2_d = nc.dram_tensor("w2r", [P, MF, D], F16, kind="ExternalInput").ap()
    out_d = nc.dram_tensor("out", [C, D], F32, kind="ExternalOutput").ap()

    with tile.TileContext(nc) as tc:
        with (
            tc.tile_pool(name="w1f", bufs=1) as w1_pool,
            tc.tile_pool(name="w2f", bufs=1) as w2_pool,
            tc.tile_pool(name="b1", bufs=1) as b1_pool,
            tc.tile_pool(name="zt", bufs=1) as z_pool,
            tc.tile_pool(name="xt", bufs=2) as xt_pool,
            tc.tile_pool(name="ht", bufs=1) as ht_pool,
            tc.tile_pool(name="ev", bufs=4) as ev_pool,
            tc.tile_pool(name="ps1", bufs=4, space="PSUM") as ps1_pool,
            tc.tile_pool(name="ps2", bufs=4, space="PSUM") as ps2_pool,
        ):
            # PE warm-up: short matmuls on a zeroed tile bridge the DMA
            # head so HAM is un-throttled when the real stream starts; at
            # N=128 the queue drains every ~110ns, so the first real
            # matmul slips in almost as soon as its data lands.
            zt = z_pool.tile([P, 512], F16)
            nc.gpsimd.memset(zt[:], 0.0)
            for _ in range(NWARM):
                psw = ps2_pool.tile([P, 512], F32, tag="ps2")
                nc.tensor.matmul(
                    psw[:, 0:128], zt[:, 0:128], zt[:, 0:128], start=True, stop=True
                )

            b1t = b1_pool.tile([P, MF], F32)
            nc.scalar.dma_start(b1t[:], b1_d[:])

            xt0 = xt_pool.tile([P, KD, 512], F16, tag="xt")
            nc.sync.dma_start(xt0[:, :, 0:256], xT_d[:, :, 0:256])
            nc.sync.dma_start(xt0[:, :, 256:512], xT_d[:, :, 256:512])

            # w1 column pieces: scalar ring brings cols 0-1023 (sized so
            # the first matmul group can start after ~1MB total lands),
            # sync ring follows with cols 1024-4095 behind xt0.
            w1f = w1_pool.tile([P, KD, F], F16)
            nc.scalar.dma_start(w1f[:, :, ds(0, 128)], w1_d[:, :, ds(0, 128)])
            nc.scalar.dma_start(w1f[:, :, ds(128, 384)], w1_d[:, :, ds(128, 384)])
            nc.scalar.dma_start(w1f[:, :, ds(512, 512)], w1_d[:, :, ds(512, 512)])
            for pi in range(2, 8):
                nc.sync.dma_start(
                    w1f[:, :, ds(pi * 512, 512)], w1_d[:, :, ds(pi * 512, 512)]
                )

            w2f = w2_pool.tile([P, MF, D], F16)

            def mm1_group(ps_sl, w1_col, xt_sl, ht_sl, j):
                for k in range(KD):
                    nc.tensor.matmul(
                        ps_sl,
                        w1f[:, k, w1_col],
                        xt_sl[k],
                        start=(k == 0),
                        stop=(k == KD - 1),
                    )
                nc.scalar.activation(ht_sl, ps_sl, GELU, bias=b1t[:, j : j + 1])

            xts = [xt0, None, None, None]
            for cn in range(CN):
                xt = xts[cn]
                ht = ht_pool.tile([P, MF, 512], F16, tag="ht")
                if cn == 0:
                    # two N=256 half-passes; w2 pieces ride the scalar
                    # ring between pass-B gelu evictions
                    for h in range(2):
                        for j in range(MF):
                            ps = ps1_pool.tile([P, 512], F32, tag="ps1")
                            mm1_group(
                                ps[:, 0:256],
                                ds(j * P, P),
                                [xt[:, k, ds(h * 256, 256)] for k in range(KD)],
                                ht[:, j, ds(h * 256, 256)],
                                j,
                            )
                            if h == 1 and j % 2 == 1:
                                p = j // 2
                                nc.scalar.dma_start(
                                    w2f[:, ds(p * 2, 2), :], w2_d[:, ds(p * 2, 2), :]
                                )
                else:
                    for j in range(MF):
                        ps = ps1_pool.tile([P, 512], F32, tag="ps1")
                        mm1_group(
                            ps[:],
                            ds(j * P, P),
                            [xt[:, k, :] for k in range(KD)],
                            ht[:, j, :],
                            j,
                        )
                # prefetch next chunk's tokens ahead of this chunk's out
                # stores in the sync queue (FIFO per engine)
                if cn + 1 < CN:
                    t = xt_pool.tile([P, KD, 512], F16, tag="xt")
                    nc.sync.dma_start(t[:], xT_d[:, :, ds((cn + 1) * 512, 512)])
                    xts[cn + 1] = t
                for cj in range(CJ):
                    row = cn * 512 + cj * P
                    for dn in range(DN):
                        ps = ps2_pool.tile([P, 512], F32, tag="ps2")
                        for j in range(MF):
                            nc.tensor.matmul(
                                ps[:],
                                ht[:, j, ds(cj * P, P)],
                                w2f[:, j, ds(dn * 512, 512)],
                                start=(j == 0),
                                stop=(j == MF - 1),
                            )
                        ev = ev_pool.tile([P, 512], F32, tag="ev")
                        last = cn == CN - 1 and cj == CJ - 1 and dn == DN - 1
                        if last:
                            # strip-pipelined eviction shortens the tail
                            for s in range(4):
                                sl = ds(s * 128, 128)
                                nc.vector.tensor_copy(ev[:, sl], ps[:, sl])
                                nc.sync.dma_start(
                                    out_d[row : row + P, ds(dn * 512 + s * 128, 128)],
                                    ev[:, sl],
                                )
                        else:
                            nc.vector.tensor_copy(ev[:], ps[:])
                            nc.sync.dma_start(
                                out_d[row : row + P, dn * 512 : (dn + 1) * 512],
                                ev[:],
                            )

    nc.compile()
    return nc


def _get_nc():
    if "nc" not in _CACHE:
        _CACHE["nc"] = _build()
    return _CACHE["nc"]


def _in_map(x_e, w1_e, b1_e, w2_e):
    xT = np.ascontiguousarray(x_e.T.reshape(KD, P, C).transpose(1, 0, 2)).astype(
        np.float16
    )
    w1r = np.ascontiguousarray(w1_e.reshape(KD, P, F).transpose(1, 0, 2)).astype(
        np.float16
    )
    b1t = np.ascontiguousarray(b1_e.reshape(MF, P).T)
    w2r = np.ascontiguousarray(w2_e.reshape(MF, P, D).transpose(1, 0, 2)).astype(
        np.float16
    )
    return {"xT": xT, "w1r": w1r, "b1t": b1t, "w2r": w2r}


def kernel(inputs, w1, b1, w2, b2, _trace=False):
    nc = _get_nc()
    x = np.asarray(inputs, dtype=np.float32).reshape(E, C, D)
    in_maps = [
        _in_map(
            x[e],
            np.asarray(w1[e], dtype=np.float32),
            np.asarray(b1[e], dtype=np.float32),
            np.asarray(w2[e], dtype=np.float32),
        )
        for e in range(E)
    ]
    res = run_bass_kernel_spmd(nc, in_maps, list(range(E)), trace=_trace)
    out = np.stack([res.results[e]["out"] for e in range(E)])[None]
    out = out + np.asarray(b2, dtype=np.float32)[None]
    if _trace:
        _CACHE["last_results"] = res
    return out.astype(np.float32)


# revision 5
# speedup vs baseline: 1.0222x; 1.0222x over previous
"""Fully-fused fp16 MoE expert FFN (E=8, C=2048, D=1024, F=4096), 8 TRN2 cores.

One expert per core; w1 AND w2 SBUF-resident in fp16. v4 schedule:
- All DMA on the two HWDGE rings (sync + scalar); both drain concurrently
  across the 16 shared SDMA engines, so aggregate reaches the HBM limit.
- w1 is repacked host-side into piece-major layout [16, P, KD, 256] and
  SBUF keeps the same order, so every 0.5MB piece moves as 4KB-contiguous
  runs on both sides (sub-1KB runs halve per-engine DMA throughput).
- Chunk 0 runs as two N=256 half-passes over dedicated contiguous token
  halves: the first matmul needs only ~1MB of DMA (xa + w1 piece 0).
- w2 (8MB) streams on the sync ring after the chunk-1 prefetch, landing
  ~10us before chunk-0 mm2 consumes it.
- N=128 warm-up matmuls on a zeroed tile bridge the DMA head so the HAM
  clock gate stays at 8/8 when the real stream starts.
"""

import numpy as np

import concourse.bass as bass
import concourse.mybir as mybir
import concourse.tile as tile
from concourse import bacc
from concourse.bass_utils import run_bass_kernel_spmd

E, C, D, F = 8, 2048, 1024, 4096
P = 128
KD = D // P  # 8
MF = F // P  # 32
CN = C // 512  # 4 chunks of 512 tokens
CJ = 4  # 128-token subblocks per chunk
DN = D // 512  # 2
NP = 16  # w1 column pieces of 256
NWARM = 34

F32 = mybir.dt.float32
F16 = mybir.dt.float16
GELU = mybir.ActivationFunctionType.Gelu_apprx_tanh
ds = bass.ds

_CACHE = {}


def _build():
    nc = bacc.Bacc("TRN2", target_bir_lowering=False, debug=False, num_devices=E)

    xa_d = nc.dram_tensor("xa", [P, KD, 256], F16, kind="ExternalInput").ap()
    xb_d = nc.dram_tensor("xb", [P, KD, 256], F16, kind="ExternalInput").ap()
    xT_d = nc.dram_tensor("xT", [P, KD, C], F16, kind="ExternalInput").ap()
    w1_d = nc.dram_tensor("w1r", [NP, P, KD, 256], F16, kind="ExternalInput").ap()
    b1_d = nc.dram_tensor("b1t", [P, MF], F32, kind="ExternalInput").ap()
    w2_d = nc.dram_tensor("w2r", [P, MF, D], F16, kind="ExternalInput").ap()
    out_d = nc.dram_tensor("out", [C, D], F32, kind="ExternalOutput").ap()

    with tile.TileContext(nc) as tc:
        with (
            tc.tile_pool(name="w1f", bufs=1) as w1_pool,
            tc.tile_pool(name="w2f", bufs=1) as w2_pool,
            tc.tile_pool(name="b1", bufs=1) as b1_pool,
            tc.tile_pool(name="zt", bufs=1) as z_pool,
            tc.tile_pool(name="xh", bufs=2) as xh_pool,
            tc.tile_pool(name="xt", bufs=2) as xt_pool,
            tc.tile_pool(name="ht", bufs=1) as ht_pool,
            tc.tile_pool(name="ev", bufs=4) as ev_pool,
            tc.tile_pool(name="ps1", bufs=4, space="PSUM") as ps1_pool,
            tc.tile_pool(name="ps2", bufs=4, space="PSUM") as ps2_pool,
        ):
            # PE warm-up bridging the DMA head (HAM stays un-throttled);
            # at N=128 the queue drains every ~110ns so the first real
            # matmul slips in as soon as its data lands.
            zt = z_pool.tile([P, 512], F16)
            nc.gpsimd.memset(zt[:], 0.0)
            for _ in range(NWARM):
                psw = ps2_pool.tile([P, 512], F32, tag="ps2")
                nc.tensor.matmul(
                    psw[:, 0:128], zt[:, 0:128], zt[:, 0:128], start=True, stop=True
                )

            b1t = b1_pool.tile([P, MF], F32)
            nc.scalar.dma_start(b1t[:], b1_d[:])

            xa = xh_pool.tile([P, KD, 256], F16, tag="xh")
            xb = xh_pool.tile([P, KD, 256], F16, tag="xh")
            nc.sync.dma_start(xa[:], xa_d[:])
            nc.sync.dma_start(xb[:], xb_d[:])

            # w1 pieces: scalar brings j-cols 0-1023 (pass-A consumes one
            # 0.25MB column block per 0.87us; both rings together sustain
            # it), sync follows with the rest behind the token halves.
            w1f = w1_pool.tile([P, NP, KD, 256], F16)
            for pc in range(4):
                nc.scalar.dma_start(w1f[:, pc, :, :], w1_d[pc])
            for pc in range(4, NP):
                nc.sync.dma_start(w1f[:, pc, :, :], w1_d[pc])

            xt1 = xt_pool.tile([P, KD, 512], F16, tag="xt")
            nc.sync.dma_start(xt1[:], xT_d[:, :, ds(512, 512)])
            xts = [None, xt1, None, None]

            # w2 after the chunk-1 prefetch on sync: issues from ~30us,
            # lands well before chunk-0 mm2 reads it (~70us).
            w2f = w2_pool.tile([P, MF, D], F16)
            for g in range(8):
                nc.sync.dma_start(
                    w2f[:, ds(g * 4, 4), :], w2_d[:, ds(g * 4, 4), :]
                )

            def w1ap(j, k):
                return w1f[:, j // 2, k, ds((j % 2) * P, P)]

            for cn in range(CN):
                ht = ht_pool.tile([P, MF, 512], F16, tag="ht")
                if cn == 0:
                    for h, xth in enumerate((xa, xb)):
                        for j in range(MF):
                            ps = ps1_pool.tile([P, 512], F32, tag="ps1")
                            for k in range(KD):
                                nc.tensor.matmul(
                                    ps[:, 0:256],
                                    w1ap(j, k),
                                    xth[:, k, :],
                                    start=(k == 0),
                                    stop=(k == KD - 1),
                                )
                            nc.scalar.activation(
                                ht[:, j, ds(h * 256, 256)],
                                ps[:, 0:256],
                                GELU,
                                bias=b1t[:, j : j + 1],
                            )
                else:
                    xt = xts[cn]
                    for j in range(MF):
                        ps = ps1_pool.tile([P, 512], F32, tag="ps1")
                        for k in range(KD):
                            nc.tensor.matmul(
                                ps[:],
                                w1ap(j, k),
                                xt[:, k, :],
                                start=(k == 0),
                                stop=(k == KD - 1),
                            )
                        nc.scalar.activation(
                            ht[:, j, :], ps[:], GELU, bias=b1t[:, j : j + 1]
                        )
                # prefetch chunk cn+2 ahead of this chunk's out stores in
                # the sync queue (chunk 1 was prefetched before w2)
                if cn + 2 < CN:
                    t = xt_pool.tile([P, KD, 512], F16, tag="xt")
                    nc.sync.dma_start(t[:], xT_d[:, :, ds((cn + 2) * 512, 512)])
                    xts[cn + 2] = t
                for cj in range(CJ):
                    row = cn * 512 + cj * P
                    for dn in range(DN):
                        ps = ps2_pool.tile([P, 512], F32, tag="ps2")
                        for j in range(MF):
                            nc.tensor.matmul(
                                ps[:],
                                ht[:, j, ds(cj * P, P)],
                                w2f[:, j, ds(dn * 512, 512)],
                                start=(j == 0),
                                stop=(j == MF - 1),
                            )
                        ev = ev_pool.tile([P, 512], F32, tag="ev")
                        nc.vector.tensor_copy(ev[:], ps[:])
                        nc.sync.dma_start(
                            out_d[row : row + P, dn * 512 : (dn + 1) * 512],
                            ev[:],
                        )

    nc.compile()
    return nc


def _get_nc():
    if "nc" not in _CACHE:
        _CACHE["nc"] = _build()
    return _CACHE["nc"]


def _in_map(x_e, w1_e, b1_e, w2_e):
    xT = np.ascontiguousarray(x_e.T.reshape(KD, P, C).transpose(1, 0, 2)).astype(
        np.float16
    )
    xa = np.ascontiguousarray(xT[:, :, 0:256])
    xb = np.ascontiguousarray(xT[:, :, 256:512])
    w1r = np.ascontiguousarray(
        w1_e.reshape(KD, P, NP, 256).transpose(2, 1, 0, 3)
    ).astype(np.float16)
    b1t = np.ascontiguousarray(b1_e.reshape(MF, P).T)
    w2r = np.ascontiguousarray(w2_e.reshape(MF, P, D).transpose(1, 0, 2)).astype(
        np.float16
    )
    return {"xa": xa, "xb": xb, "xT": xT, "w1r": w1r, "b1t": b1t, "w2r": w2r}


def kernel(inputs, w1, b1, w2, b2, _trace=False):
    nc = _get_nc()
    x = np.asarray(inputs, dtype=np.float32).reshape(E, C, D)
    in_maps = [
        _in_map(
            x[e],
            np.asarray(w1[e], dtype=np.float32),
            np.asarray(b1[e], dtype=np.float32),
            np.asarray(w2[e], dtype=np.float32),
        )
        for e in range(E)
    ]
    res = run_bass_kernel_spmd(nc, in_maps, list(range(E)), trace=_trace)
    out = np.stack([res.results[e]["out"] for e in range(E)])[None]
    out = out + np.asarray(b2, dtype=np.float32)[None]
    if _trace:
        _CACHE["last_results"] = res
    return out.astype(np.float32)
